# revision 71
# baseline (speedup 1.0000x reference)
"""Multi-head attention Trainium2 kernel (8 NeuronCores, tensor-parallel over heads).

Strategy:
  - 16 heads / 8 cores = 2 heads per core. x is replicated; Wq/Wk/Wv sharded by
    head; Wp row-sharded (contraction dim). Each core computes a partial
    projection output [B*T, D]; the host sums the 8 partials (+bias).
  - On chip, all contractions need the contracted dim on SBUF partitions, so the
    host passes xT = x.reshape(BT, D).T and per-core transposed weight slices.
  - qT/kT are computed packed [128 = 2 heads x 64, BT]. Scores are computed
    transposed (s on partitions, t on free) so softmax normalization can ride
    the attn@v matmul: lhsT = [v | ones] gives out rows 0..63 = unnormalized
    out^T and row 64 = the softmax denominator Z. Softmax is computed without
    max subtraction (scores are O(10), exp stays in fp32 range). 1/Z is
    replicated across partitions by a K=1 matmul with a ones row (Pool's
    partition_broadcast reads physical partition 0 regardless of the AP and
    GPSIMD cannot read PSUM on HW), then multiplied in on DVE. Each block's
    normalize is deferred into the NEXT block's filler stream so the bc
    matmul never stalls the PE on the reciprocal.
  - Causality: only lower-triangular [128s x 512t] blocks are computed; blocks
    straddling the diagonal narrow their scores/av matmuls to the unmasked
    column range [dd:TBLK] and apply one shared [128,2,128] staircase mask
    (multiplicative, after exp, both heads in one op).
  - Both heads' score tiles share one 2-bank PSUM tile so each i-step does a
    single [128,2,w] exp instruction. The attention i-loop alone is ACT-paced
    (exp ~1000ns/step vs 854ns of PE work), so the next block's QKV matmuls
    and the previous block's projection are interleaved into the i-loop as
    small filler units: the PE always has independent work while ACT runs exp.
    attn@v lags the scores by TWO i-steps so exp latency and the previous
    block's normalize chain (which must free the av PSUM ring slots) stay off
    the PE critical path.
  - Precision: every matmul operand lives in a float32r (TF32-like) tile.
    The BIR verifier requires f32r matmul inputs to be *produced* rounded, so
    DRAM inputs are declared f32r (DMA is a byte copy; the PE rounds on read)
    and on-chip producers (PSUM->SBUF copies, exp, muls) write f32r directly.
    Memset/iota on f32r tiles fails ISA codegen, so constants are built in f32
    scratch and round-copied. f32r matmul is 4x faster than f32 at N>=256.
  - Engine balance: only DVE/ACT can read PSUM on HW. All steady-state
    PSUM->SBUF drains go to DVE -- an ACT copy would sit between exps in
    ACT's in-order queue and stall the attention i-loop. Pool handles
    SBUF-only work (staircase masking). The last block pipelines its
    normalize+proj+store per 128-column stripe (nothing is left to hide the
    serial chain under); PE is the overall bottleneck at ~129us busy.
"""

import numpy as np

B, T, D, H, HD = 2, 2048, 1024, 16, 64
NCORES = 8
HPC = H // NCORES          # heads per core = 2
CH = HPC * HD              # channels per core = 128
BT = B * T

_CACHE = {}


def _build(b, t, d, use_f32r):
    """Build + compile the per-core Bass program."""
    import concourse.tile as tile
    from concourse import bacc, mybir
    from concourse.masks import make_identity
    from contextlib import ExitStack

    f32 = mybir.dt.float32
    dtt = mybir.dt.float32r if use_f32r else f32

    bt = b * t
    KT = d // 128            # k-tiles over the model dim
    TBLK = min(512, t)       # t-block width for scores/attn
    NJ = t // TBLK           # t-blocks per batch
    NSB = bt // 128          # 128-row s-blocks over B*T
    SPT = TBLK // 128        # s-blocks per t-block

    nc = bacc.Bacc("TRN2", target_bir_lowering=False, debug=False)

    bf16 = mybir.dt.bfloat16
    xT = nc.dram_tensor("xT", [d, bt], dtt, kind="ExternalInput").ap()
    wq = nc.dram_tensor("wq", [d, CH], dtt, kind="ExternalInput").ap()
    wk = nc.dram_tensor("wk", [d, CH], dtt, kind="ExternalInput").ap()
    wv = nc.dram_tensor("wv", [d, CH], dtt, kind="ExternalInput").ap()
    wp = nc.dram_tensor("wp", [CH, d], dtt, kind="ExternalInput").ap()
    # bf16 partials halve the store traffic; the host upcasts and sums in
    # fp32, so only one rounding of each partial is added (~1e-3 rel)
    out_p = nc.dram_tensor("out_p", [bt, d], bf16, kind="ExternalOutput").ap()
    # per-(block,head) scratch rows for the 1/Z partition broadcast: a DMA
    # can replicate a DRAM row across all partitions (0-stride source dim),
    # which SBUF-side APs cannot express
    zscr = nc.dram_tensor("zscr", [b * NJ * HPC, TBLK], f32, kind="Internal").ap()

    with tile.TileContext(nc) as tc, ExitStack() as top:
        persist = top.enter_context(tc.tile_pool(name="persist", bufs=1))

        # ---- persistent tiles ----
        qT_sb = persist.tile([128, bt], dtt, tag="qT")
        kT_sb = persist.tile([128, bt], dtt, tag="kT")
        # [v_h0 | 1 | pad | v_h1 | 1 | pad] per 128-row s-block
        vaug = persist.tile([128, NSB, 66 * HPC], dtt, tag="vaug")
        outT_sb = persist.tile([128, bt], dtt, tag="outT")
        wq_sb = persist.tile([128, KT, CH], dtt, tag="wq")
        wk_sb = persist.tile([128, KT, CH], dtt, tag="wk")
        wv_sb = persist.tile([128, KT, CH], dtt, tag="wv")
        wp_sb = persist.tile([128, d], dtt, tag="wp")
        ident = persist.tile([128, 128], dtt, tag="ident")
        mask2 = persist.tile([128, HPC, 128], dtt, tag="mask2")
        ones1 = persist.tile([65, HD], dtt, tag="ones1")

        # constants are built in f32 scratch (memset/iota on f32r fails ISA
        # codegen) and round-copied into their f32r homes; the scratch lives
        # in the persistent pool -- a dedicated pool's exit barrier would
        # stall the first x-tile allocations behind the constant copies
        ident_f = persist.tile([128, 128], f32, tag="ident_f")
        mask_f = persist.tile([128, 128], f32, tag="mask_f")
        ones_f = persist.tile([128, HD], f32, tag="ones_f")

        def emit_consts():
            make_identity(nc, ident_f[:])
            nc.gpsimd.memset(ones_f[:], 1.0)
            # staircase mask: keep (p <= c), upper-triangular incl. diagonal
            nc.gpsimd.memset(mask_f[:], 1.0)
            nc.gpsimd.affine_select(
                out=mask_f[:], in_=mask_f[:],
                compare_op=mybir.AluOpType.is_ge,
                fill=0.0, base=0,
                # iota = -p + c ; keep when >= 0
                pattern=[[1, 128]], channel_multiplier=-1,
            )
            nc.vector.tensor_copy(ident[:], ident_f[:])
            for h in range(HPC):
                nc.vector.tensor_copy(mask2[:, h, :], mask_f[:])
            nc.vector.tensor_copy(ones1[:], ones_f[0:65, :])
            # ones column of vaug (col 64 of each 66-wide head group)
            for sb in range(NSB):
                nc.vector.tensor_copy(
                    vaug[:, sb, :].rearrange(
                        "p (g c) -> p g c", g=HPC)[:, :, 64:65],
                    ones_f[:, 0:HPC].rearrange("p (g c) -> p g c", g=HPC),
                )

        # ---- merged loop: per (batch, t-block): QKV -> attention -> proj ----
        # Attention for block j of batch bb needs q columns of block j and
        # k/v columns of blocks 0..j (same batch) -- all computed by the time
        # block j's QKV is done. The next block's QKV and the previous block's
        # projection are emitted as small units interleaved into the current
        # block's attention i-loop (see module docstring).
        PW = min(512, d)
        NIB = d // PW
        with ExitStack() as body:
            xpool = body.enter_context(tc.tile_pool(name="xpool", bufs=3))
            vtpool = body.enter_context(tc.tile_pool(name="vtpool", bufs=2))
            npool = body.enter_context(tc.tile_pool(name="npool", bufs=8))
            zpool = body.enter_context(tc.tile_pool(name="zpool", bufs=2))
            tmpool = body.enter_context(tc.tile_pool(name="tmpool", bufs=4))
            opool = body.enter_context(tc.tile_pool(name="opool", bufs=4))
            # PSUM budget (8 banks): qkv/transpose/proj ring 2 + score pairs
            # (2 banks each) 4 + av 2
            ps_mix = body.enter_context(tc.tile_pool(name="ps_mix", bufs=2, space="PSUM"))
            ps_s = body.enter_context(tc.tile_pool(name="ps_s", bufs=2, space="PSUM"))
            ps_av = body.enter_context(tc.tile_pool(name="ps_av", bufs=2, space="PSUM"))

            def emit_xt_load(bb, j):
                """Per-k-tile x loads: consumers wait only for their slice."""
                col0 = bb * t + j * TBLK
                tsl = slice(col0, col0 + TBLK)
                xt = xpool.tile([128, KT, TBLK], dtt, tag="xt", name=f"xt_{bb}_{j}")
                for kt in range(KT):
                    nc.sync.dma_start(
                        out=xt[:, kt, :],
                        in_=xT[kt * 128:(kt + 1) * 128, tsl],
                    )
                return xt

            def emit_qkv_units(bb, j, xt, first=False):
                """Generator: one small PE bundle per next() call."""
                col0 = bb * t + j * TBLK
                tsl = slice(col0, col0 + TBLK)
                # the qT drain goes to ACT: at block handoff DVE is busy with
                # the normalize chain, and the next block's scores wait on
                # exactly these copies (Pool/GPSIMD cannot read PSUM on HW)
                for w_sb, dst, cp in ((wq_sb, qT_sb, nc.vector.tensor_copy),
                                      (wk_sb, kT_sb, nc.vector.tensor_copy)):
                    ps = ps_mix.tile([128, TBLK], f32, tag="ps_mix",
                                     name=f"psq_{bb}_{j}_{dst.name}")
                    for kt in range(KT):
                        nc.tensor.matmul(ps[:], w_sb[:, kt, :], xt[:, kt, :],
                                         start=(kt == 0), stop=(kt == KT - 1))
                        yield
                    cp(dst[:, tsl], ps[:])
                    yield
                ps = ps_mix.tile([128, TBLK], f32, tag="ps_mix", name=f"psv_{bb}_{j}")
                for kt in range(KT):
                    nc.tensor.matmul(ps[:], wv_sb[:, kt, :], xt[:, kt, :],
                                     start=(kt == 0), stop=(kt == KT - 1))
                    yield
                vt = vtpool.tile([128, TBLK], dtt, tag="vt", name=f"vt_{bb}_{j}")
                nc.vector.tensor_copy(vt[:], ps[:])
                yield
                # all 4 v^T transposes land in one PSUM bank, drained by a
                # single strided DVE copy into vaug
                pt = ps_mix.tile([128, SPT, 128], f32, tag="ps_mix",
                                 name=f"ptr_{bb}_{j}")
                for s4 in range(SPT):
                    sub = pt[:, s4, :]
                    pout = sub.bitcast(dtt) if use_f32r else sub
                    nc.tensor.transpose(pout, vt[:, s4 * 128:(s4 + 1) * 128], ident[:])
                    yield
                sb0 = col0 // 128
                nc.vector.tensor_copy(
                    vaug[:, sb0:sb0 + SPT, :].rearrange(
                        "p s (g c) -> p s g c", g=HPC)[:, :, :, 0:HD],
                    pt[:].rearrange("p s (g c) -> p s g c", g=HPC),
                )
                yield
                if first:
                    nc.gpsimd.dma_start(out=wp_sb[:], in_=wp)

            def emit_proj_units(bb, j, use_act=False):
                """Generator: one proj matmul (+drain/store) per next().
                use_act: the consuming attention block is short (few exps),
                so ACT can absorb half the PSUM drains without stalling."""
                col0 = bb * t + j * TBLK
                for tl in range(TBLK // 128):
                    tt = col0 // 128 + tl
                    ot = opool.tile([128, NIB, PW], bf16, tag="ot",
                                    name=f"ot_{bb}_{j}_{tl}")
                    for ib in range(NIB):
                        ps = ps_mix.tile([128, PW], f32, tag="ps_mix",
                                         name=f"psp_{bb}_{j}_{tl}_{ib}")
                        nc.tensor.matmul(ps[:], outT_sb[:, tt * 128:(tt + 1) * 128],
                                         wp_sb[:, ib * PW:(ib + 1) * PW],
                                         start=True, stop=True)
                        # PSUM->SBUF drains: DVE by default (an ACT copy
                        # between exps stalls the loop), ACT for half when
                        # the consuming block is exp-light
                        if use_act and (tl * NIB + ib) % 2 == 1:
                            nc.scalar.copy(ot[:, ib, :], ps[:])
                        else:
                            nc.vector.tensor_copy(ot[:, ib, :], ps[:])
                        yield
                    nc.sync.dma_start(
                        out=out_p[tt * 128:(tt + 1) * 128, :], in_=ot[:])
                    yield

            def emit_attn(bb, j, filler, striped_tail=False):
                col0 = bb * t + j * TBLK
                n_i = (j + 1) * SPT
                avs = [ps_av.tile([65, TBLK], f32, tag="ps_av", name=f"av_{bb}_{j}_{h}")
                       for h in range(HPC)]

                def emit_av(i_, nh_, lo_):
                    for h in range(HPC):
                        sb_idx = (bb * t + i_ * 128) // 128
                        nc.tensor.matmul(
                            avs[h][:, lo_:TBLK],
                            vaug[:, sb_idx, h * 66:h * 66 + HD + 1],
                            nh_[:, h, lo_:TBLK],
                            start=(i_ == 0), stop=(i_ == n_i - 1),
                            skip_group_check=True)

                pend = []           # (i, nh, lo) waiting for their av matmuls
                for i in range(n_i):
                    filler_ok = i >= 1   # let prefetched deps land first
                    ssl = slice(bb * t + i * 128, bb * t + i * 128 + 128)
                    dd = 128 * i - TBLK * j
                    lo = max(dd, 0)
                    # both heads' scores in one 2-bank PSUM tile -> one exp
                    ps = ps_s.tile([128, HPC, TBLK], f32, tag="ps_s",
                                   name=f"pss_{bb}_{j}_{i}")
                    for h in range(HPC):
                        hp = slice(h * HD, (h + 1) * HD)
                        nc.tensor.matmul(
                            ps[:, h, lo:TBLK], kT_sb[hp, ssl],
                            qT_sb[hp, col0 + lo:col0 + TBLK],
                            start=True, stop=True)
                    nh = npool.tile([128, HPC, TBLK], dtt, tag="nh",
                                    name=f"nh_{bb}_{j}_{i}")
                    nc.scalar.activation(
                        nh[:, :, lo:TBLK], ps[:, :, lo:TBLK],
                        mybir.ActivationFunctionType.Exp, scale=0.125)
                    if dd >= 0:
                        # SBUF-only op: Pool is legal here and otherwise idle
                        nc.gpsimd.tensor_mul(
                            nh[:, :, dd:dd + 128], nh[:, :, dd:dd + 128], mask2[:])
                    # independent PE work (next block's QKV / prev block's
                    # proj) fills the gap while ACT computes this exp
                    if filler_ok:
                        for _f in range(8):
                            next(filler, None)
                    # attn@v lags TWO i-steps: exp latency and the previous
                    # block's normalize never stall the PE
                    pend.append((i, nh, lo))
                    if len(pend) > 2:
                        emit_av(*pend.pop(0))
                for p_ in pend:
                    emit_av(*p_)
                # flush remaining filler before the serial normalize chain
                for _ in filler:
                    pass

                def emit_recips(sfx, c0=0, cw=TBLK):
                    # 1/Z lives on partition 64; reciprocal is pure DVE work
                    # and never touches the PE stream
                    rws = []
                    for h in range(HPC):
                        rrow = zpool.tile([65, TBLK], dtt, tag="rrow",
                                          name=f"rr_{bb}_{j}_{h}{sfx}")
                        with nc.allow_low_precision(reason="1/Z bcast via f32r matmul"):
                            nc.vector.reciprocal(rrow[64:65, c0:c0 + cw],
                                                 avs[h][64:65, c0:c0 + cw])
                        rws.append(rrow)
                    return rws

                def norm_units(rws, c0, cw, sfx, via_pe=False):
                    """Scale out^T columns [c0, c0+cw) of this block by 1/Z.
                    Deferred (mid-kernel) normalizes replicate 1/Z across the
                    64 partitions with a DRAM round-trip DMA -- no PE or DVE
                    work, and the latency hides in the next block's filler.
                    The tail keeps the K=1 ones-row matmul: the PE is idle
                    there while two extra DMA hops would stretch the exposed
                    chain. (Pool's partition_broadcast op reads physical
                    partition 0 regardless of AP, and SBUF APs reject stride-0
                    partitions, so those paths are out.)"""
                    csl = slice(col0 + c0, col0 + c0 + cw)
                    for h in range(HPC):
                        bcs = tmpool.tile([HD, TBLK], f32, tag="bcs",
                                          name=f"bcs_{bb}_{j}_{h}{sfx}")
                        if via_pe:
                            bc = ps_mix.tile([HD, TBLK], f32, tag="ps_mix",
                                             name=f"bc_{bb}_{j}_{h}{sfx}")
                            nc.tensor.matmul(bc[:, 0:cw], ones1[64:65, :],
                                             rws[h][64:65, c0:c0 + cw],
                                             start=True, stop=True)
                            nc.scalar.copy(bcs[:, 0:cw], bc[:, 0:cw])
                        else:
                            zrow = (bb * NJ + j) * HPC + h
                            nc.sync.dma_start(
                                out=zscr[zrow, c0:c0 + cw],
                                in_=rws[h][64:65, c0:c0 + cw].bitcast(f32))
                            nc.sync.dma_start(
                                out=bcs[:, 0:cw],
                                in_=zscr[zrow, c0:c0 + cw].partition_broadcast(HD))
                        if h == 0:
                            nc.vector.tensor_mul(outT_sb[0:HD, csl],
                                                 avs[h][0:HD, c0:c0 + cw], bcs[:, 0:cw])
                        else:
                            # DVE lanes cannot shift partitions; route rows
                            # 64:128 through an SBUF->SBUF DMA
                            tmp = tmpool.tile([HD, TBLK], dtt, tag="tmp",
                                              name=f"tm_{bb}_{j}{sfx}")
                            nc.vector.tensor_mul(tmp[:, 0:cw],
                                                 avs[h][0:HD, c0:c0 + cw], bcs[:, 0:cw])
                            nc.sync.dma_start(
                                out=outT_sb[h * HD:(h + 1) * HD, csl],
                                in_=tmp[:, 0:cw])
                        yield

                if not striped_tail:
                    # defer the PE-touching normalize into the next block's
                    # filler: the bc matmul then never stalls on the recip
                    rws = emit_recips("")
                    return norm_units(rws, 0, TBLK, "")
                # final block: nothing left to hide the serial normalize ->
                # proj -> store chain under, so pipeline it per 128-col
                # stripe. One full-width reciprocal pass; bc rides the (now
                # otherwise idle) ps_mix ring while the proj pairs use the
                # freed 2-bank ps_s slots, so the stripes don't serialize on
                # a shared PSUM ring.
                rws = emit_recips("L")
                for tl in range(TBLK // 128):
                    for _ in norm_units(rws, tl * 128, 128, f"s{tl}",
                                        via_pe=True):
                        pass
                    tt = col0 // 128 + tl
                    ot = opool.tile([128, NIB, PW], bf16, tag="ot",
                                    name=f"otL_{bb}_{j}_{tl}")
                    ps = ps_s.tile([128, NIB, PW], f32, tag="ps_s",
                                   name=f"pspL_{bb}_{j}_{tl}")
                    for ib in range(NIB):
                        nc.tensor.matmul(ps[:, ib, :],
                                         outT_sb[:, tt * 128:(tt + 1) * 128],
                                         wp_sb[:, ib * PW:(ib + 1) * PW],
                                         start=True, stop=True)
                        if ib % 2 == 0:
                            nc.scalar.copy(ot[:, ib, :], ps[:, ib, :])
                        else:
                            nc.vector.tensor_copy(ot[:, ib, :], ps[:, ib, :])
                    nc.sync.dma_start(
                        out=out_p[tt * 128:(tt + 1) * 128, :], in_=ot[:])

            # software pipeline: consume block 0's QKV eagerly, then each
            # attention interleaves the next QKV + previous proj as filler
            import itertools
            blocks = [(bb, j) for bb in range(b) for j in range(NJ)]
            # startup is one long serial DMA stretch; emission order is the
            # scheduler's priority order, so sequence the transfers the way
            # the first matmuls consume them: x k-tile 0, Wq, remaining x,
            # Wk, Wv (Wp only matters ~40us in)
            bb0, j0 = blocks[0]
            tsl0 = slice(bb0 * t + j0 * TBLK, bb0 * t + j0 * TBLK + TBLK)
            xt0 = xpool.tile([128, KT, TBLK], dtt, tag="xt", name="xt_first")
            nc.sync.dma_start(out=xt0[:, 0, :], in_=xT[0:128, tsl0])
            nc.sync.dma_start(
                out=wq_sb[:], in_=wq.rearrange("(kt p) m -> p kt m", p=128))
            for kt in range(1, KT):
                nc.sync.dma_start(
                    out=xt0[:, kt, :], in_=xT[kt * 128:(kt + 1) * 128, tsl0])
            nc.sync.dma_start(
                out=wk_sb[:], in_=wk.rearrange("(kt p) m -> p kt m", p=128))
            nc.sync.dma_start(
                out=wv_sb[:], in_=wv.rearrange("(kt p) m -> p kt m", p=128))
            emit_consts()
            for _ in emit_qkv_units(*blocks[0], xt0, first=True):
                pass
            norm_prev = None
            for idx, blk in enumerate(blocks):
                last = idx + 1 == len(blocks)
                fill = []
                if norm_prev is not None:
                    fill.append(norm_prev)     # prev block's deferred 1/Z
                if not last:
                    xt_n = emit_xt_load(*blocks[idx + 1])
                    fill.append(emit_qkv_units(*blocks[idx + 1], xt_n))
                if idx >= 1:
                    fill.append(emit_proj_units(*blocks[idx - 1],
                                                use_act=blk[1] <= 1))
                norm_prev = emit_attn(*blk, filler=itertools.chain(*fill),
                                      striped_tail=last)

    nc.compile()
    return nc


def _get_nc(b=B, t=T, d=D, cfg="f32r"):
    key = (b, t, d, cfg)
    if key not in _CACHE:
        _CACHE[key] = _build(b, t, d, cfg == "f32r")
    return _CACHE[key]


def _prepare_in_maps(x, Wq, Wk, Wv, Wp, b, t, d, n_heads):
    bt = b * t
    xT = np.ascontiguousarray(x.reshape(bt, d).T.astype(np.float32))
    in_maps = []
    for c in range(NCORES):
        h0 = c * HPC
        wq_c = np.ascontiguousarray(Wq[h0:h0 + HPC].reshape(CH, d).T.astype(np.float32))
        wk_c = np.ascontiguousarray(Wk[h0:h0 + HPC].reshape(CH, d).T.astype(np.float32))
        wv_c = np.ascontiguousarray(Wv[h0:h0 + HPC].reshape(CH, d).T.astype(np.float32))
        wp_c = np.ascontiguousarray(Wp[:, c * CH:(c + 1) * CH].T.astype(np.float32))
        in_maps.append({"xT": xT, "wq": wq_c, "wk": wk_c, "wv": wv_c, "wp": wp_c})
    return in_maps


def _run(x, Wq, Wk, Wv, Wp, bp, b, t, d, cfg, trace=False):
    from concourse.bass_utils import run_bass_kernel_spmd
    nc = _get_nc(b, t, d, cfg)
    in_maps = _prepare_in_maps(x, Wq, Wk, Wv, Wp, b, t, d, H)
    res = run_bass_kernel_spmd(nc, in_maps, core_ids=list(range(NCORES)), trace=trace)
    acc = np.zeros((b * t, d), dtype=np.float64)
    for r in res.results:
        acc += r["out_p"].astype(np.float64)
    out = (acc + np.asarray(bp, dtype=np.float64)).astype(np.float32)
    return out.reshape(b, t, d), res


KERNEL_CFG = "f32r"


def kernel(x, Wq, Wk, Wv, Wp, bp):
    out, _ = _run(np.asarray(x), np.asarray(Wq), np.asarray(Wk), np.asarray(Wv),
                  np.asarray(Wp), np.asarray(bp), B, T, D, KERNEL_CFG, trace=False)
    return out


# revision 73
# speedup vs baseline: 1.0008x; 1.0008x over previous
"""Multi-head attention Trainium2 kernel (8 NeuronCores, tensor-parallel over heads).

Strategy:
  - 16 heads / 8 cores = 2 heads per core. x is replicated; Wq/Wk/Wv sharded by
    head; Wp row-sharded (contraction dim). Each core computes a partial
    projection output [B*T, D]; the host sums the 8 partials (+bias).
  - On chip, all contractions need the contracted dim on SBUF partitions, so the
    host passes xT = x.reshape(BT, D).T and per-core transposed weight slices.
  - qT/kT are computed packed [128 = 2 heads x 64, BT]. Scores are computed
    transposed (s on partitions, t on free) so softmax normalization can ride
    the attn@v matmul: lhsT = [v | ones] gives out rows 0..63 = unnormalized
    out^T and row 64 = the softmax denominator Z. Softmax is computed without
    max subtraction (scores are O(10), exp stays in fp32 range). 1/Z is
    replicated across partitions by a K=1 matmul with a ones row (Pool's
    partition_broadcast reads physical partition 0 regardless of the AP and
    GPSIMD cannot read PSUM on HW), then multiplied in on DVE. Each block's
    normalize is deferred into the NEXT block's filler stream so the bc
    matmul never stalls the PE on the reciprocal.
  - Causality: only lower-triangular [128s x 512t] blocks are computed; blocks
    straddling the diagonal narrow their scores/av matmuls to the unmasked
    column range [dd:TBLK] and apply one shared [128,2,128] staircase mask
    (multiplicative, after exp, both heads in one op).
  - Both heads' score tiles share one 2-bank PSUM tile so each i-step does a
    single [128,2,w] exp instruction. The attention i-loop alone is ACT-paced
    (exp ~1000ns/step vs 854ns of PE work), so the next block's QKV matmuls
    and the previous block's projection are interleaved into the i-loop as
    small filler units: the PE always has independent work while ACT runs exp.
    attn@v lags the scores by TWO i-steps so exp latency and the previous
    block's normalize chain (which must free the av PSUM ring slots) stay off
    the PE critical path.
  - Precision: every matmul operand lives in a float32r (TF32-like) tile.
    The BIR verifier requires f32r matmul inputs to be *produced* rounded, so
    DRAM inputs are declared f32r (DMA is a byte copy; the PE rounds on read)
    and on-chip producers (PSUM->SBUF copies, exp, muls) write f32r directly.
    Memset/iota on f32r tiles fails ISA codegen, so constants are built in f32
    scratch and round-copied. f32r matmul is 4x faster than f32 at N>=256.
  - Engine balance: only DVE/ACT can read PSUM on HW. All steady-state
    PSUM->SBUF drains go to DVE -- an ACT copy would sit between exps in
    ACT's in-order queue and stall the attention i-loop. Pool handles
    SBUF-only work (staircase masking). The last block pipelines its
    normalize+proj+store per 128-column stripe (nothing is left to hide the
    serial chain under); PE is the overall bottleneck at ~129us busy.
"""

import numpy as np

B, T, D, H, HD = 2, 2048, 1024, 16, 64
NCORES = 8
HPC = H // NCORES          # heads per core = 2
CH = HPC * HD              # channels per core = 128
BT = B * T

_CACHE = {}


def _build(b, t, d, use_f32r):
    """Build + compile the per-core Bass program."""
    import concourse.tile as tile
    from concourse import bacc, mybir
    from concourse.masks import make_identity
    from contextlib import ExitStack

    f32 = mybir.dt.float32
    dtt = mybir.dt.float32r if use_f32r else f32

    bt = b * t
    KT = d // 128            # k-tiles over the model dim
    TBLK = min(512, t)       # t-block width for scores/attn
    NJ = t // TBLK           # t-blocks per batch
    NSB = bt // 128          # 128-row s-blocks over B*T
    SPT = TBLK // 128        # s-blocks per t-block

    nc = bacc.Bacc("TRN2", target_bir_lowering=False, debug=False)

    bf16 = mybir.dt.bfloat16
    xT = nc.dram_tensor("xT", [d, bt], dtt, kind="ExternalInput").ap()
    wq = nc.dram_tensor("wq", [d, CH], dtt, kind="ExternalInput").ap()
    wk = nc.dram_tensor("wk", [d, CH], dtt, kind="ExternalInput").ap()
    wv = nc.dram_tensor("wv", [d, CH], dtt, kind="ExternalInput").ap()
    wp = nc.dram_tensor("wp", [CH, d], dtt, kind="ExternalInput").ap()
    # bf16 partials halve the store traffic; the host upcasts and sums in
    # fp32, so only one rounding of each partial is added (~1e-3 rel)
    out_p = nc.dram_tensor("out_p", [bt, d], bf16, kind="ExternalOutput").ap()
    # per-(block,head) scratch rows for the 1/Z partition broadcast: a DMA
    # can replicate a DRAM row across all partitions (0-stride source dim),
    # which SBUF-side APs cannot express
    zscr = nc.dram_tensor("zscr", [b * NJ * HPC, TBLK], f32, kind="Internal").ap()

    with tile.TileContext(nc) as tc, ExitStack() as top:
        persist = top.enter_context(tc.tile_pool(name="persist", bufs=1))

        # ---- persistent tiles ----
        qT_sb = persist.tile([128, bt], dtt, tag="qT")
        kT_sb = persist.tile([128, bt], dtt, tag="kT")
        # [v_h0 | 1 | pad | v_h1 | 1 | pad] per 128-row s-block
        vaug = persist.tile([128, NSB, 66 * HPC], dtt, tag="vaug")
        outT_sb = persist.tile([128, bt], dtt, tag="outT")
        wq_sb = persist.tile([128, KT, CH], dtt, tag="wq")
        wk_sb = persist.tile([128, KT, CH], dtt, tag="wk")
        wv_sb = persist.tile([128, KT, CH], dtt, tag="wv")
        wp_sb = persist.tile([128, d], dtt, tag="wp")
        ident = persist.tile([128, 128], dtt, tag="ident")
        mask2 = persist.tile([128, HPC, 128], dtt, tag="mask2")
        ones1 = persist.tile([65, HD], dtt, tag="ones1")

        # constants are built in f32 scratch (memset/iota on f32r fails ISA
        # codegen) and round-copied into their f32r homes; the scratch lives
        # in the persistent pool -- a dedicated pool's exit barrier would
        # stall the first x-tile allocations behind the constant copies
        ident_f = persist.tile([128, 128], f32, tag="ident_f")
        mask_f = persist.tile([128, 128], f32, tag="mask_f")
        ones_f = persist.tile([128, HD], f32, tag="ones_f")

        def emit_consts():
            make_identity(nc, ident_f[:])
            nc.gpsimd.memset(ones_f[:], 1.0)
            # staircase mask: keep (p <= c), upper-triangular incl. diagonal
            nc.gpsimd.memset(mask_f[:], 1.0)
            nc.gpsimd.affine_select(
                out=mask_f[:], in_=mask_f[:],
                compare_op=mybir.AluOpType.is_ge,
                fill=0.0, base=0,
                # iota = -p + c ; keep when >= 0
                pattern=[[1, 128]], channel_multiplier=-1,
            )
            nc.vector.tensor_copy(ident[:], ident_f[:])
            # preload the Exp activation table while the PE waits on startup
            # DMAs; ident_f is dead after the copy above, so the written cell
            # is harmless (writing a LIVE constant here corrupts the kernel)
            nc.scalar.activation(ident_f[0:1, 0:1], ident_f[0:1, 0:1],
                                 mybir.ActivationFunctionType.Exp, scale=1.0)
            for h in range(HPC):
                nc.vector.tensor_copy(mask2[:, h, :], mask_f[:])
            nc.vector.tensor_copy(ones1[:], ones_f[0:65, :])
            # ones column of vaug (col 64 of each 66-wide head group)
            for sb in range(NSB):
                nc.vector.tensor_copy(
                    vaug[:, sb, :].rearrange(
                        "p (g c) -> p g c", g=HPC)[:, :, 64:65],
                    ones_f[:, 0:HPC].rearrange("p (g c) -> p g c", g=HPC),
                )

        # ---- merged loop: per (batch, t-block): QKV -> attention -> proj ----
        # Attention for block j of batch bb needs q columns of block j and
        # k/v columns of blocks 0..j (same batch) -- all computed by the time
        # block j's QKV is done. The next block's QKV and the previous block's
        # projection are emitted as small units interleaved into the current
        # block's attention i-loop (see module docstring).
        PW = min(512, d)
        NIB = d // PW
        with ExitStack() as body:
            xpool = body.enter_context(tc.tile_pool(name="xpool", bufs=3))
            vtpool = body.enter_context(tc.tile_pool(name="vtpool", bufs=2))
            npool = body.enter_context(tc.tile_pool(name="npool", bufs=8))
            zpool = body.enter_context(tc.tile_pool(name="zpool", bufs=2))
            tmpool = body.enter_context(tc.tile_pool(name="tmpool", bufs=4))
            opool = body.enter_context(tc.tile_pool(name="opool", bufs=4))
            # PSUM budget (8 banks): qkv/transpose/proj ring 2 + score pairs
            # (2 banks each) 4 + av 2
            ps_mix = body.enter_context(tc.tile_pool(name="ps_mix", bufs=2, space="PSUM"))
            ps_s = body.enter_context(tc.tile_pool(name="ps_s", bufs=2, space="PSUM"))
            ps_av = body.enter_context(tc.tile_pool(name="ps_av", bufs=2, space="PSUM"))

            def emit_xt_load(bb, j):
                """Per-k-tile x loads: consumers wait only for their slice."""
                col0 = bb * t + j * TBLK
                tsl = slice(col0, col0 + TBLK)
                xt = xpool.tile([128, KT, TBLK], dtt, tag="xt", name=f"xt_{bb}_{j}")
                for kt in range(KT):
                    nc.sync.dma_start(
                        out=xt[:, kt, :],
                        in_=xT[kt * 128:(kt + 1) * 128, tsl],
                    )
                return xt

            def emit_qkv_units(bb, j, xt, first=False):
                """Generator: one small PE bundle per next() call."""
                col0 = bb * t + j * TBLK
                tsl = slice(col0, col0 + TBLK)
                # the qT drain goes to ACT: at block handoff DVE is busy with
                # the normalize chain, and the next block's scores wait on
                # exactly these copies (Pool/GPSIMD cannot read PSUM on HW)
                for w_sb, dst, cp in ((wq_sb, qT_sb, nc.vector.tensor_copy),
                                      (wk_sb, kT_sb, nc.vector.tensor_copy)):
                    ps = ps_mix.tile([128, TBLK], f32, tag="ps_mix",
                                     name=f"psq_{bb}_{j}_{dst.name}")
                    for kt in range(KT):
                        nc.tensor.matmul(ps[:], w_sb[:, kt, :], xt[:, kt, :],
                                         start=(kt == 0), stop=(kt == KT - 1))
                        yield
                    cp(dst[:, tsl], ps[:])
                    yield
                ps = ps_mix.tile([128, TBLK], f32, tag="ps_mix", name=f"psv_{bb}_{j}")
                for kt in range(KT):
                    nc.tensor.matmul(ps[:], wv_sb[:, kt, :], xt[:, kt, :],
                                     start=(kt == 0), stop=(kt == KT - 1))
                    yield
                vt = vtpool.tile([128, TBLK], dtt, tag="vt", name=f"vt_{bb}_{j}")
                nc.vector.tensor_copy(vt[:], ps[:])
                yield
                # all 4 v^T transposes land in one PSUM bank, drained by a
                # single strided DVE copy into vaug
                pt = ps_mix.tile([128, SPT, 128], f32, tag="ps_mix",
                                 name=f"ptr_{bb}_{j}")
                for s4 in range(SPT):
                    sub = pt[:, s4, :]
                    pout = sub.bitcast(dtt) if use_f32r else sub
                    nc.tensor.transpose(pout, vt[:, s4 * 128:(s4 + 1) * 128], ident[:])
                    yield
                sb0 = col0 // 128
                nc.vector.tensor_copy(
                    vaug[:, sb0:sb0 + SPT, :].rearrange(
                        "p s (g c) -> p s g c", g=HPC)[:, :, :, 0:HD],
                    pt[:].rearrange("p s (g c) -> p s g c", g=HPC),
                )
                yield
                if first:
                    nc.gpsimd.dma_start(out=wp_sb[:], in_=wp)

            def emit_proj_units(bb, j, use_act=False):
                """Generator: one proj matmul (+drain/store) per next().
                use_act: the consuming attention block is short (few exps),
                so ACT can absorb half the PSUM drains without stalling."""
                col0 = bb * t + j * TBLK
                for tl in range(TBLK // 128):
                    tt = col0 // 128 + tl
                    ot = opool.tile([128, NIB, PW], bf16, tag="ot",
                                    name=f"ot_{bb}_{j}_{tl}")
                    for ib in range(NIB):
                        ps = ps_mix.tile([128, PW], f32, tag="ps_mix",
                                         name=f"psp_{bb}_{j}_{tl}_{ib}")
                        nc.tensor.matmul(ps[:], outT_sb[:, tt * 128:(tt + 1) * 128],
                                         wp_sb[:, ib * PW:(ib + 1) * PW],
                                         start=True, stop=True)
                        # PSUM->SBUF drains: DVE by default (an ACT copy
                        # between exps stalls the loop), ACT for half when
                        # the consuming block is exp-light
                        if use_act and (tl * NIB + ib) % 2 == 1:
                            nc.scalar.copy(ot[:, ib, :], ps[:])
                        else:
                            nc.vector.tensor_copy(ot[:, ib, :], ps[:])
                        yield
                    nc.sync.dma_start(
                        out=out_p[tt * 128:(tt + 1) * 128, :], in_=ot[:])
                    yield

            def emit_attn(bb, j, filler, striped_tail=False):
                col0 = bb * t + j * TBLK
                n_i = (j + 1) * SPT
                avs = [ps_av.tile([65, TBLK], f32, tag="ps_av", name=f"av_{bb}_{j}_{h}")
                       for h in range(HPC)]

                def emit_av(i_, nh_, lo_):
                    for h in range(HPC):
                        sb_idx = (bb * t + i_ * 128) // 128
                        nc.tensor.matmul(
                            avs[h][:, lo_:TBLK],
                            vaug[:, sb_idx, h * 66:h * 66 + HD + 1],
                            nh_[:, h, lo_:TBLK],
                            start=(i_ == 0), stop=(i_ == n_i - 1),
                            skip_group_check=True)

                pend = []           # (i, nh, lo) waiting for their av matmuls
                for i in range(n_i):
                    filler_ok = i >= 1   # let prefetched deps land first
                    ssl = slice(bb * t + i * 128, bb * t + i * 128 + 128)
                    dd = 128 * i - TBLK * j
                    lo = max(dd, 0)
                    # both heads' scores in one 2-bank PSUM tile -> one exp
                    ps = ps_s.tile([128, HPC, TBLK], f32, tag="ps_s",
                                   name=f"pss_{bb}_{j}_{i}")
                    for h in range(HPC):
                        hp = slice(h * HD, (h + 1) * HD)
                        nc.tensor.matmul(
                            ps[:, h, lo:TBLK], kT_sb[hp, ssl],
                            qT_sb[hp, col0 + lo:col0 + TBLK],
                            start=True, stop=True)
                    nh = npool.tile([128, HPC, TBLK], dtt, tag="nh",
                                    name=f"nh_{bb}_{j}_{i}")
                    nc.scalar.activation(
                        nh[:, :, lo:TBLK], ps[:, :, lo:TBLK],
                        mybir.ActivationFunctionType.Exp, scale=0.125)
                    if dd >= 0:
                        # SBUF-only op: Pool is legal here and otherwise idle
                        nc.gpsimd.tensor_mul(
                            nh[:, :, dd:dd + 128], nh[:, :, dd:dd + 128], mask2[:])
                    # independent PE work (next block's QKV / prev block's
                    # proj) fills the gap while ACT computes this exp
                    if filler_ok:
                        for _f in range(8):
                            next(filler, None)
                    # attn@v lags TWO i-steps: exp latency and the previous
                    # block's normalize never stall the PE
                    pend.append((i, nh, lo))
                    if len(pend) > 2:
                        emit_av(*pend.pop(0))
                for p_ in pend:
                    emit_av(*p_)
                # flush remaining filler before the serial normalize chain
                for _ in filler:
                    pass

                def emit_recips(sfx, c0=0, cw=TBLK):
                    # 1/Z lives on partition 64; reciprocal is pure DVE work
                    # and never touches the PE stream
                    rws = []
                    for h in range(HPC):
                        rrow = zpool.tile([65, TBLK], dtt, tag="rrow",
                                          name=f"rr_{bb}_{j}_{h}{sfx}")
                        with nc.allow_low_precision(reason="1/Z bcast via f32r matmul"):
                            nc.vector.reciprocal(rrow[64:65, c0:c0 + cw],
                                                 avs[h][64:65, c0:c0 + cw])
                        rws.append(rrow)
                    return rws

                def norm_units(rws, c0, cw, sfx, via_pe=False):
                    """Scale out^T columns [c0, c0+cw) of this block by 1/Z.
                    Deferred (mid-kernel) normalizes replicate 1/Z across the
                    64 partitions with a DRAM round-trip DMA -- no PE or DVE
                    work, and the latency hides in the next block's filler.
                    The tail keeps the K=1 ones-row matmul: the PE is idle
                    there while two extra DMA hops would stretch the exposed
                    chain. (Pool's partition_broadcast op reads physical
                    partition 0 regardless of AP, and SBUF APs reject stride-0
                    partitions, so those paths are out.)"""
                    csl = slice(col0 + c0, col0 + c0 + cw)
                    for h in range(HPC):
                        bcs = tmpool.tile([HD, TBLK], f32, tag="bcs",
                                          name=f"bcs_{bb}_{j}_{h}{sfx}")
                        if via_pe:
                            bc = ps_mix.tile([HD, TBLK], f32, tag="ps_mix",
                                             name=f"bc_{bb}_{j}_{h}{sfx}")
                            nc.tensor.matmul(bc[:, 0:cw], ones1[64:65, :],
                                             rws[h][64:65, c0:c0 + cw],
                                             start=True, stop=True)
                            nc.scalar.copy(bcs[:, 0:cw], bc[:, 0:cw])
                        else:
                            zrow = (bb * NJ + j) * HPC + h
                            nc.sync.dma_start(
                                out=zscr[zrow, c0:c0 + cw],
                                in_=rws[h][64:65, c0:c0 + cw].bitcast(f32))
                            nc.sync.dma_start(
                                out=bcs[:, 0:cw],
                                in_=zscr[zrow, c0:c0 + cw].partition_broadcast(HD))
                        if h == 0:
                            nc.vector.tensor_mul(outT_sb[0:HD, csl],
                                                 avs[h][0:HD, c0:c0 + cw], bcs[:, 0:cw])
                        else:
                            # DVE lanes cannot shift partitions; route rows
                            # 64:128 through an SBUF->SBUF DMA
                            tmp = tmpool.tile([HD, TBLK], dtt, tag="tmp",
                                              name=f"tm_{bb}_{j}{sfx}")
                            nc.vector.tensor_mul(tmp[:, 0:cw],
                                                 avs[h][0:HD, c0:c0 + cw], bcs[:, 0:cw])
                            nc.sync.dma_start(
                                out=outT_sb[h * HD:(h + 1) * HD, csl],
                                in_=tmp[:, 0:cw])
                        yield

                if not striped_tail:
                    # defer the PE-touching normalize into the next block's
                    # filler: the bc matmul then never stalls on the recip
                    rws = emit_recips("")
                    return norm_units(rws, 0, TBLK, "")
                # final block: nothing left to hide the serial normalize ->
                # proj -> store chain under, so pipeline it per 128-col
                # stripe. One full-width reciprocal pass; bc rides the (now
                # otherwise idle) ps_mix ring while the proj pairs use the
                # freed 2-bank ps_s slots, so the stripes don't serialize on
                # a shared PSUM ring.
                rws = emit_recips("L")
                for tl in range(TBLK // 128):
                    for _ in norm_units(rws, tl * 128, 128, f"s{tl}",
                                        via_pe=True):
                        pass
                    tt = col0 // 128 + tl
                    ot = opool.tile([128, NIB, PW], bf16, tag="ot",
                                    name=f"otL_{bb}_{j}_{tl}")
                    ps = ps_s.tile([128, NIB, PW], f32, tag="ps_s",
                                   name=f"pspL_{bb}_{j}_{tl}")
                    for ib in range(NIB):
                        nc.tensor.matmul(ps[:, ib, :],
                                         outT_sb[:, tt * 128:(tt + 1) * 128],
                                         wp_sb[:, ib * PW:(ib + 1) * PW],
                                         start=True, stop=True)
                        if ib % 2 == 0:
                            nc.scalar.copy(ot[:, ib, :], ps[:, ib, :])
                        else:
                            nc.vector.tensor_copy(ot[:, ib, :], ps[:, ib, :])
                    nc.sync.dma_start(
                        out=out_p[tt * 128:(tt + 1) * 128, :], in_=ot[:])

            # software pipeline: consume block 0's QKV eagerly, then each
            # attention interleaves the next QKV + previous proj as filler
            import itertools
            blocks = [(bb, j) for bb in range(b) for j in range(NJ)]
            # startup is one long serial DMA stretch; emission order is the
            # scheduler's priority order, so sequence the transfers the way
            # the first matmuls consume them: x k-tile 0, Wq, remaining x,
            # Wk, Wv (Wp only matters ~40us in)
            bb0, j0 = blocks[0]
            tsl0 = slice(bb0 * t + j0 * TBLK, bb0 * t + j0 * TBLK + TBLK)
            xt0 = xpool.tile([128, KT, TBLK], dtt, tag="xt", name="xt_first")
            nc.sync.dma_start(out=xt0[:, 0, :], in_=xT[0:128, tsl0])
            nc.sync.dma_start(
                out=wq_sb[:], in_=wq.rearrange("(kt p) m -> p kt m", p=128))
            for kt in range(1, KT):
                nc.sync.dma_start(
                    out=xt0[:, kt, :], in_=xT[kt * 128:(kt + 1) * 128, tsl0])
            nc.sync.dma_start(
                out=wk_sb[:], in_=wk.rearrange("(kt p) m -> p kt m", p=128))
            nc.sync.dma_start(
                out=wv_sb[:], in_=wv.rearrange("(kt p) m -> p kt m", p=128))
            emit_consts()
            for _ in emit_qkv_units(*blocks[0], xt0, first=True):
                pass
            norm_prev = None
            for idx, blk in enumerate(blocks):
                last = idx + 1 == len(blocks)
                fill = []
                if norm_prev is not None:
                    fill.append(norm_prev)     # prev block's deferred 1/Z
                if not last:
                    xt_n = emit_xt_load(*blocks[idx + 1])
                    fill.append(emit_qkv_units(*blocks[idx + 1], xt_n))
                if idx >= 1:
                    fill.append(emit_proj_units(*blocks[idx - 1],
                                                use_act=blk[1] <= 1))
                norm_prev = emit_attn(*blk, filler=itertools.chain(*fill),
                                      striped_tail=last)

    nc.compile()
    return nc


def _get_nc(b=B, t=T, d=D, cfg="f32r"):
    key = (b, t, d, cfg)
    if key not in _CACHE:
        _CACHE[key] = _build(b, t, d, cfg == "f32r")
    return _CACHE[key]


def _prepare_in_maps(x, Wq, Wk, Wv, Wp, b, t, d, n_heads):
    bt = b * t
    xT = np.ascontiguousarray(x.reshape(bt, d).T.astype(np.float32))
    in_maps = []
    for c in range(NCORES):
        h0 = c * HPC
        wq_c = np.ascontiguousarray(Wq[h0:h0 + HPC].reshape(CH, d).T.astype(np.float32))
        wk_c = np.ascontiguousarray(Wk[h0:h0 + HPC].reshape(CH, d).T.astype(np.float32))
        wv_c = np.ascontiguousarray(Wv[h0:h0 + HPC].reshape(CH, d).T.astype(np.float32))
        wp_c = np.ascontiguousarray(Wp[:, c * CH:(c + 1) * CH].T.astype(np.float32))
        in_maps.append({"xT": xT, "wq": wq_c, "wk": wk_c, "wv": wv_c, "wp": wp_c})
    return in_maps


def _run(x, Wq, Wk, Wv, Wp, bp, b, t, d, cfg, trace=False):
    from concourse.bass_utils import run_bass_kernel_spmd
    nc = _get_nc(b, t, d, cfg)
    in_maps = _prepare_in_maps(x, Wq, Wk, Wv, Wp, b, t, d, H)
    res = run_bass_kernel_spmd(nc, in_maps, core_ids=list(range(NCORES)), trace=trace)
    acc = np.zeros((b * t, d), dtype=np.float64)
    for r in res.results:
        acc += r["out_p"].astype(np.float64)
    out = (acc + np.asarray(bp, dtype=np.float64)).astype(np.float32)
    return out.reshape(b, t, d), res


KERNEL_CFG = "f32r"


def kernel(x, Wq, Wk, Wv, Wp, bp):
    out, _ = _run(np.asarray(x), np.asarray(Wq), np.asarray(Wk), np.asarray(Wv),
                  np.asarray(Wp), np.asarray(bp), B, T, D, KERNEL_CFG, trace=False)
    return out


# revision 75
# speedup vs baseline: 1.0009x; 1.0001x over previous
"""Multi-head attention Trainium2 kernel (8 NeuronCores, tensor-parallel over heads).

Strategy:
  - 16 heads / 8 cores = 2 heads per core. x is replicated; Wq/Wk/Wv sharded by
    head; Wp row-sharded (contraction dim). Each core computes a partial
    projection output [B*T, D]; the host sums the 8 partials (+bias).
  - On chip, all contractions need the contracted dim on SBUF partitions, so the
    host passes xT = x.reshape(BT, D).T and per-core transposed weight slices.
  - qT/kT are computed packed [128 = 2 heads x 64, BT]. Scores are computed
    transposed (s on partitions, t on free) so softmax normalization can ride
    the attn@v matmul: lhsT = [v | ones] gives out rows 0..63 = unnormalized
    out^T and row 64 = the softmax denominator Z. Softmax is computed without
    max subtraction (scores are O(10), exp stays in fp32 range). 1/Z is
    replicated across partitions by a K=1 matmul with a ones row (Pool's
    partition_broadcast reads physical partition 0 regardless of the AP and
    GPSIMD cannot read PSUM on HW), then multiplied in on DVE. Each block's
    normalize is deferred into the NEXT block's filler stream so the bc
    matmul never stalls the PE on the reciprocal.
  - Causality: only lower-triangular [128s x 512t] blocks are computed; blocks
    straddling the diagonal narrow their scores/av matmuls to the unmasked
    column range [dd:TBLK] and apply one shared [128,2,128] staircase mask
    (multiplicative, after exp, both heads in one op).
  - Both heads' score tiles share one 2-bank PSUM tile so each i-step does a
    single [128,2,w] exp instruction. The attention i-loop alone is ACT-paced
    (exp ~1000ns/step vs 854ns of PE work), so the next block's QKV matmuls
    and the previous block's projection are interleaved into the i-loop as
    small filler units: the PE always has independent work while ACT runs exp.
    attn@v lags the scores by TWO i-steps so exp latency and the previous
    block's normalize chain (which must free the av PSUM ring slots) stay off
    the PE critical path.
  - Precision: every matmul operand lives in a float32r (TF32-like) tile.
    The BIR verifier requires f32r matmul inputs to be *produced* rounded, so
    DRAM inputs are declared f32r (DMA is a byte copy; the PE rounds on read)
    and on-chip producers (PSUM->SBUF copies, exp, muls) write f32r directly.
    Memset/iota on f32r tiles fails ISA codegen, so constants are built in f32
    scratch and round-copied. f32r matmul is 4x faster than f32 at N>=256.
  - Engine balance: only DVE/ACT can read PSUM on HW. All steady-state
    PSUM->SBUF drains go to DVE -- an ACT copy would sit between exps in
    ACT's in-order queue and stall the attention i-loop. Pool handles
    SBUF-only work (staircase masking). The last block pipelines its
    normalize+proj+store per 128-column stripe (nothing is left to hide the
    serial chain under); PE is the overall bottleneck at ~129us busy.
"""

import numpy as np

B, T, D, H, HD = 2, 2048, 1024, 16, 64
NCORES = 8
HPC = H // NCORES          # heads per core = 2
CH = HPC * HD              # channels per core = 128
BT = B * T

_CACHE = {}


def _build(b, t, d, use_f32r):
    """Build + compile the per-core Bass program."""
    import concourse.tile as tile
    from concourse import bacc, mybir
    from concourse.masks import make_identity
    from contextlib import ExitStack

    f32 = mybir.dt.float32
    dtt = mybir.dt.float32r if use_f32r else f32

    bt = b * t
    KT = d // 128            # k-tiles over the model dim
    TBLK = min(512, t)       # t-block width for scores/attn
    NJ = t // TBLK           # t-blocks per batch
    NSB = bt // 128          # 128-row s-blocks over B*T
    SPT = TBLK // 128        # s-blocks per t-block

    nc = bacc.Bacc("TRN2", target_bir_lowering=False, debug=False)

    bf16 = mybir.dt.bfloat16
    xT = nc.dram_tensor("xT", [d, bt], dtt, kind="ExternalInput").ap()
    wq = nc.dram_tensor("wq", [d, CH], dtt, kind="ExternalInput").ap()
    wk = nc.dram_tensor("wk", [d, CH], dtt, kind="ExternalInput").ap()
    wv = nc.dram_tensor("wv", [d, CH], dtt, kind="ExternalInput").ap()
    wp = nc.dram_tensor("wp", [CH, d], dtt, kind="ExternalInput").ap()
    # bf16 partials halve the store traffic; the host upcasts and sums in
    # fp32, so only one rounding of each partial is added (~1e-3 rel)
    out_p = nc.dram_tensor("out_p", [bt, d], bf16, kind="ExternalOutput").ap()
    # per-(block,head) scratch rows for the 1/Z partition broadcast: a DMA
    # can replicate a DRAM row across all partitions (0-stride source dim),
    # which SBUF-side APs cannot express
    zscr = nc.dram_tensor("zscr", [b * NJ * HPC, TBLK], f32, kind="Internal").ap()

    with tile.TileContext(nc) as tc, ExitStack() as top:
        persist = top.enter_context(tc.tile_pool(name="persist", bufs=1))

        # ---- persistent tiles ----
        qT_sb = persist.tile([128, bt], dtt, tag="qT")
        kT_sb = persist.tile([128, bt], dtt, tag="kT")
        # [v_h0 | 1 | pad | v_h1 | 1 | pad] per 128-row s-block
        vaug = persist.tile([128, NSB, 66 * HPC], dtt, tag="vaug")
        outT_sb = persist.tile([128, bt], dtt, tag="outT")
        wq_sb = persist.tile([128, KT, CH], dtt, tag="wq")
        wk_sb = persist.tile([128, KT, CH], dtt, tag="wk")
        wv_sb = persist.tile([128, KT, CH], dtt, tag="wv")
        wp_sb = persist.tile([128, d], dtt, tag="wp")
        ident = persist.tile([128, 128], dtt, tag="ident")
        mask2 = persist.tile([128, HPC, 128], dtt, tag="mask2")
        ones1 = persist.tile([65, HD], dtt, tag="ones1")

        # constants are built in f32 scratch (memset/iota on f32r fails ISA
        # codegen) and round-copied into their f32r homes; the scratch lives
        # in the persistent pool -- a dedicated pool's exit barrier would
        # stall the first x-tile allocations behind the constant copies
        ident_f = persist.tile([128, 128], f32, tag="ident_f")
        mask_f = persist.tile([128, 128], f32, tag="mask_f")
        ones_f = persist.tile([128, HD], f32, tag="ones_f")

        def emit_consts():
            make_identity(nc, ident_f[:])
            nc.gpsimd.memset(ones_f[:], 1.0)
            # staircase mask: keep (p <= c), upper-triangular incl. diagonal
            nc.gpsimd.memset(mask_f[:], 1.0)
            nc.gpsimd.affine_select(
                out=mask_f[:], in_=mask_f[:],
                compare_op=mybir.AluOpType.is_ge,
                fill=0.0, base=0,
                # iota = -p + c ; keep when >= 0
                pattern=[[1, 128]], channel_multiplier=-1,
            )
            nc.vector.tensor_copy(ident[:], ident_f[:])
            # preload the Exp activation table while the PE waits on startup
            # DMAs; ident_f is dead after the copy above, so the written cell
            # is harmless (writing a LIVE constant here corrupts the kernel)
            nc.scalar.activation(ident_f[0:1, 0:1], ident_f[0:1, 0:1],
                                 mybir.ActivationFunctionType.Exp, scale=1.0)
            for h in range(HPC):
                nc.vector.tensor_copy(mask2[:, h, :], mask_f[:])
            nc.vector.tensor_copy(ones1[:], ones_f[0:65, :])
            # ones column of vaug (col 64 of each 66-wide head group)
            for sb in range(NSB):
                nc.vector.tensor_copy(
                    vaug[:, sb, :].rearrange(
                        "p (g c) -> p g c", g=HPC)[:, :, 64:65],
                    ones_f[:, 0:HPC].rearrange("p (g c) -> p g c", g=HPC),
                )

        # ---- merged loop: per (batch, t-block): QKV -> attention -> proj ----
        # Attention for block j of batch bb needs q columns of block j and
        # k/v columns of blocks 0..j (same batch) -- all computed by the time
        # block j's QKV is done. The next block's QKV and the previous block's
        # projection are emitted as small units interleaved into the current
        # block's attention i-loop (see module docstring).
        PW = min(512, d)
        NIB = d // PW
        with ExitStack() as body:
            xpool = body.enter_context(tc.tile_pool(name="xpool", bufs=3))
            vtpool = body.enter_context(tc.tile_pool(name="vtpool", bufs=2))
            npool = body.enter_context(tc.tile_pool(name="npool", bufs=8))
            zpool = body.enter_context(tc.tile_pool(name="zpool", bufs=2))
            tmpool = body.enter_context(tc.tile_pool(name="tmpool", bufs=4))
            opool = body.enter_context(tc.tile_pool(name="opool", bufs=4))
            # PSUM budget (8 banks): qkv/transpose/proj ring 2 + score pairs
            # (2 banks each) 4 + av 2
            ps_mix = body.enter_context(tc.tile_pool(name="ps_mix", bufs=2, space="PSUM"))
            ps_s = body.enter_context(tc.tile_pool(name="ps_s", bufs=2, space="PSUM"))
            ps_av = body.enter_context(tc.tile_pool(name="ps_av", bufs=2, space="PSUM"))

            def emit_xt_load(bb, j):
                """Per-k-tile x loads: consumers wait only for their slice."""
                col0 = bb * t + j * TBLK
                tsl = slice(col0, col0 + TBLK)
                xt = xpool.tile([128, KT, TBLK], dtt, tag="xt", name=f"xt_{bb}_{j}")
                for kt in range(KT):
                    nc.sync.dma_start(
                        out=xt[:, kt, :],
                        in_=xT[kt * 128:(kt + 1) * 128, tsl],
                    )
                return xt

            def emit_qkv_units(bb, j, xt, first=False):
                """Generator: one small PE bundle per next() call."""
                col0 = bb * t + j * TBLK
                tsl = slice(col0, col0 + TBLK)
                # the qT drain goes to ACT: at block handoff DVE is busy with
                # the normalize chain, and the next block's scores wait on
                # exactly these copies (Pool/GPSIMD cannot read PSUM on HW)
                for w_sb, dst, cp in ((wq_sb, qT_sb, nc.vector.tensor_copy),
                                      (wk_sb, kT_sb, nc.vector.tensor_copy)):
                    ps = ps_mix.tile([128, TBLK], f32, tag="ps_mix",
                                     name=f"psq_{bb}_{j}_{dst.name}")
                    for kt in range(KT):
                        nc.tensor.matmul(ps[:], w_sb[:, kt, :], xt[:, kt, :],
                                         start=(kt == 0), stop=(kt == KT - 1))
                        yield
                    cp(dst[:, tsl], ps[:])
                    yield
                ps = ps_mix.tile([128, TBLK], f32, tag="ps_mix", name=f"psv_{bb}_{j}")
                for kt in range(KT):
                    nc.tensor.matmul(ps[:], wv_sb[:, kt, :], xt[:, kt, :],
                                     start=(kt == 0), stop=(kt == KT - 1))
                    yield
                vt = vtpool.tile([128, TBLK], dtt, tag="vt", name=f"vt_{bb}_{j}")
                nc.vector.tensor_copy(vt[:], ps[:])
                yield
                # all 4 v^T transposes land in one PSUM bank, drained by a
                # single strided DVE copy into vaug
                pt = ps_mix.tile([128, SPT, 128], f32, tag="ps_mix",
                                 name=f"ptr_{bb}_{j}")
                for s4 in range(SPT):
                    sub = pt[:, s4, :]
                    pout = sub.bitcast(dtt) if use_f32r else sub
                    nc.tensor.transpose(pout, vt[:, s4 * 128:(s4 + 1) * 128], ident[:])
                    yield
                sb0 = col0 // 128
                nc.vector.tensor_copy(
                    vaug[:, sb0:sb0 + SPT, :].rearrange(
                        "p s (g c) -> p s g c", g=HPC)[:, :, :, 0:HD],
                    pt[:].rearrange("p s (g c) -> p s g c", g=HPC),
                )
                yield
                if first:
                    nc.gpsimd.dma_start(out=wp_sb[:], in_=wp)

            def emit_proj_units(bb, j, use_act=False):
                """Generator: one proj matmul (+drain/store) per next().
                use_act: the consuming attention block is short (few exps),
                so ACT can absorb half the PSUM drains without stalling."""
                col0 = bb * t + j * TBLK
                for tl in range(TBLK // 128):
                    tt = col0 // 128 + tl
                    ot = opool.tile([128, NIB, PW], bf16, tag="ot",
                                    name=f"ot_{bb}_{j}_{tl}")
                    for ib in range(NIB):
                        ps = ps_mix.tile([128, PW], f32, tag="ps_mix",
                                         name=f"psp_{bb}_{j}_{tl}_{ib}")
                        nc.tensor.matmul(ps[:], outT_sb[:, tt * 128:(tt + 1) * 128],
                                         wp_sb[:, ib * PW:(ib + 1) * PW],
                                         start=True, stop=True)
                        # PSUM->SBUF drains: DVE by default (an ACT copy
                        # between exps stalls the loop), ACT for half when
                        # the consuming block is exp-light
                        if use_act and (tl * NIB + ib) % 2 == 1:
                            nc.scalar.copy(ot[:, ib, :], ps[:])
                        else:
                            nc.vector.tensor_copy(ot[:, ib, :], ps[:])
                        yield
                    nc.sync.dma_start(
                        out=out_p[tt * 128:(tt + 1) * 128, :], in_=ot[:])
                    yield

            def emit_attn(bb, j, filler, striped_tail=False):
                col0 = bb * t + j * TBLK
                n_i = (j + 1) * SPT
                avs = [ps_av.tile([65, TBLK], f32, tag="ps_av", name=f"av_{bb}_{j}_{h}")
                       for h in range(HPC)]

                def emit_av(i_, nh_, lo_):
                    for h in range(HPC):
                        sb_idx = (bb * t + i_ * 128) // 128
                        nc.tensor.matmul(
                            avs[h][:, lo_:TBLK],
                            vaug[:, sb_idx, h * 66:h * 66 + HD + 1],
                            nh_[:, h, lo_:TBLK],
                            start=(i_ == 0), stop=(i_ == n_i - 1),
                            skip_group_check=True)

                pend = []           # (i, nh, lo) waiting for their av matmuls
                for i in range(n_i):
                    filler_ok = i >= 1   # let prefetched deps land first
                    ssl = slice(bb * t + i * 128, bb * t + i * 128 + 128)
                    dd = 128 * i - TBLK * j
                    lo = max(dd, 0)
                    # both heads' scores in one 2-bank PSUM tile -> one exp
                    ps = ps_s.tile([128, HPC, TBLK], f32, tag="ps_s",
                                   name=f"pss_{bb}_{j}_{i}")
                    for h in range(HPC):
                        hp = slice(h * HD, (h + 1) * HD)
                        nc.tensor.matmul(
                            ps[:, h, lo:TBLK], kT_sb[hp, ssl],
                            qT_sb[hp, col0 + lo:col0 + TBLK],
                            start=True, stop=True)
                    nh = npool.tile([128, HPC, TBLK], dtt, tag="nh",
                                    name=f"nh_{bb}_{j}_{i}")
                    nc.scalar.activation(
                        nh[:, :, lo:TBLK], ps[:, :, lo:TBLK],
                        mybir.ActivationFunctionType.Exp, scale=0.125)
                    if dd >= 0:
                        # SBUF-only op: Pool is legal here and otherwise idle
                        nc.gpsimd.tensor_mul(
                            nh[:, :, dd:dd + 128], nh[:, :, dd:dd + 128], mask2[:])
                    # independent PE work (next block's QKV / prev block's
                    # proj) fills the gap while ACT computes this exp
                    if filler_ok:
                        for _f in range(8):
                            next(filler, None)
                    # attn@v lags TWO i-steps: exp latency and the previous
                    # block's normalize never stall the PE
                    pend.append((i, nh, lo))
                    if len(pend) > 2:
                        emit_av(*pend.pop(0))
                for p_ in pend:
                    emit_av(*p_)
                # flush remaining filler before the serial normalize chain
                for _ in filler:
                    pass

                def emit_recips(sfx, c0=0, cw=TBLK):
                    # 1/Z lives on partition 64; reciprocal is pure DVE work
                    # and never touches the PE stream
                    rws = []
                    for h in range(HPC):
                        rrow = zpool.tile([65, TBLK], dtt, tag="rrow",
                                          name=f"rr_{bb}_{j}_{h}{sfx}")
                        with nc.allow_low_precision(reason="1/Z bcast via f32r matmul"):
                            nc.vector.reciprocal(rrow[64:65, c0:c0 + cw],
                                                 avs[h][64:65, c0:c0 + cw])
                        rws.append(rrow)
                    return rws

                def norm_units(rws, c0, cw, sfx, via_pe=False):
                    """Scale out^T columns [c0, c0+cw) of this block by 1/Z.
                    Deferred (mid-kernel) normalizes replicate 1/Z across the
                    64 partitions with a DRAM round-trip DMA -- no PE or DVE
                    work, and the latency hides in the next block's filler.
                    The tail keeps the K=1 ones-row matmul: the PE is idle
                    there while two extra DMA hops would stretch the exposed
                    chain. (Pool's partition_broadcast op reads physical
                    partition 0 regardless of AP, and SBUF APs reject stride-0
                    partitions, so those paths are out.)"""
                    csl = slice(col0 + c0, col0 + c0 + cw)
                    for h in range(HPC):
                        bcs = tmpool.tile([HD, TBLK], f32, tag="bcs",
                                          name=f"bcs_{bb}_{j}_{h}{sfx}")
                        if via_pe:
                            bc = ps_mix.tile([HD, TBLK], f32, tag="ps_mix",
                                             name=f"bc_{bb}_{j}_{h}{sfx}")
                            nc.tensor.matmul(bc[:, 0:cw], ones1[64:65, :],
                                             rws[h][64:65, c0:c0 + cw],
                                             start=True, stop=True)
                            nc.scalar.copy(bcs[:, 0:cw], bc[:, 0:cw])
                        else:
                            zrow = (bb * NJ + j) * HPC + h
                            nc.sync.dma_start(
                                out=zscr[zrow, c0:c0 + cw],
                                in_=rws[h][64:65, c0:c0 + cw].bitcast(f32))
                            nc.sync.dma_start(
                                out=bcs[:, 0:cw],
                                in_=zscr[zrow, c0:c0 + cw].partition_broadcast(HD))
                        if h == 0:
                            nc.vector.tensor_mul(outT_sb[0:HD, csl],
                                                 avs[h][0:HD, c0:c0 + cw], bcs[:, 0:cw])
                        else:
                            # DVE lanes cannot shift partitions; route rows
                            # 64:128 through an SBUF->SBUF DMA
                            tmp = tmpool.tile([HD, TBLK], dtt, tag="tmp",
                                              name=f"tm_{bb}_{j}{sfx}")
                            nc.vector.tensor_mul(tmp[:, 0:cw],
                                                 avs[h][0:HD, c0:c0 + cw], bcs[:, 0:cw])
                            nc.sync.dma_start(
                                out=outT_sb[h * HD:(h + 1) * HD, csl],
                                in_=tmp[:, 0:cw])
                        yield

                if not striped_tail:
                    # defer the PE-touching normalize into the next block's
                    # filler: the bc matmul then never stalls on the recip
                    rws = emit_recips("")
                    return norm_units(rws, 0, TBLK, "")
                # final block: nothing left to hide the serial normalize ->
                # proj -> store chain under, so pipeline it per 128-col
                # stripe. One full-width reciprocal pass; bc rides the (now
                # otherwise idle) ps_mix ring while the proj pairs use the
                # freed 2-bank ps_s slots, so the stripes don't serialize on
                # a shared PSUM ring.
                rws = emit_recips("L")
                # 256-wide normalize stripes (half the chain hops); proj
                # below still consumes 128-wide pieces as they complete
                for s2 in range(TBLK // 256):
                    for _ in norm_units(rws, s2 * 256, 256, f"s{s2}",
                                        via_pe=True):
                        pass
                for tl in range(TBLK // 128):
                    tt = col0 // 128 + tl
                    ot = opool.tile([128, NIB, PW], bf16, tag="ot",
                                    name=f"otL_{bb}_{j}_{tl}")
                    ps = ps_s.tile([128, NIB, PW], f32, tag="ps_s",
                                   name=f"pspL_{bb}_{j}_{tl}")
                    for ib in range(NIB):
                        nc.tensor.matmul(ps[:, ib, :],
                                         outT_sb[:, tt * 128:(tt + 1) * 128],
                                         wp_sb[:, ib * PW:(ib + 1) * PW],
                                         start=True, stop=True)
                        if ib % 2 == 0:
                            nc.scalar.copy(ot[:, ib, :], ps[:, ib, :])
                        else:
                            nc.vector.tensor_copy(ot[:, ib, :], ps[:, ib, :])
                    nc.sync.dma_start(
                        out=out_p[tt * 128:(tt + 1) * 128, :], in_=ot[:])

            # software pipeline: consume block 0's QKV eagerly, then each
            # attention interleaves the next QKV + previous proj as filler
            import itertools
            blocks = [(bb, j) for bb in range(b) for j in range(NJ)]
            # startup is one long serial DMA stretch; emission order is the
            # scheduler's priority order, so sequence the transfers the way
            # the first matmuls consume them: x k-tile 0, Wq, remaining x,
            # Wk, Wv (Wp only matters ~40us in)
            bb0, j0 = blocks[0]
            tsl0 = slice(bb0 * t + j0 * TBLK, bb0 * t + j0 * TBLK + TBLK)
            xt0 = xpool.tile([128, KT, TBLK], dtt, tag="xt", name="xt_first")
            nc.sync.dma_start(out=xt0[:, 0, :], in_=xT[0:128, tsl0])
            nc.sync.dma_start(
                out=wq_sb[:], in_=wq.rearrange("(kt p) m -> p kt m", p=128))
            for kt in range(1, KT):
                nc.sync.dma_start(
                    out=xt0[:, kt, :], in_=xT[kt * 128:(kt + 1) * 128, tsl0])
            nc.sync.dma_start(
                out=wk_sb[:], in_=wk.rearrange("(kt p) m -> p kt m", p=128))
            nc.sync.dma_start(
                out=wv_sb[:], in_=wv.rearrange("(kt p) m -> p kt m", p=128))
            emit_consts()
            for _ in emit_qkv_units(*blocks[0], xt0, first=True):
                pass
            norm_prev = None
            for idx, blk in enumerate(blocks):
                last = idx + 1 == len(blocks)
                fill = []
                if norm_prev is not None:
                    fill.append(norm_prev)     # prev block's deferred 1/Z
                if not last:
                    xt_n = emit_xt_load(*blocks[idx + 1])
                    fill.append(emit_qkv_units(*blocks[idx + 1], xt_n))
                if idx >= 1:
                    fill.append(emit_proj_units(*blocks[idx - 1],
                                                use_act=blk[1] <= 1))
                norm_prev = emit_attn(*blk, filler=itertools.chain(*fill),
                                      striped_tail=last)

    nc.compile()
    return nc


def _get_nc(b=B, t=T, d=D, cfg="f32r"):
    key = (b, t, d, cfg)
    if key not in _CACHE:
        _CACHE[key] = _build(b, t, d, cfg == "f32r")
    return _CACHE[key]


def _prepare_in_maps(x, Wq, Wk, Wv, Wp, b, t, d, n_heads):
    bt = b * t
    xT = np.ascontiguousarray(x.reshape(bt, d).T.astype(np.float32))
    in_maps = []
    for c in range(NCORES):
        h0 = c * HPC
        wq_c = np.ascontiguousarray(Wq[h0:h0 + HPC].reshape(CH, d).T.astype(np.float32))
        wk_c = np.ascontiguousarray(Wk[h0:h0 + HPC].reshape(CH, d).T.astype(np.float32))
        wv_c = np.ascontiguousarray(Wv[h0:h0 + HPC].reshape(CH, d).T.astype(np.float32))
        wp_c = np.ascontiguousarray(Wp[:, c * CH:(c + 1) * CH].T.astype(np.float32))
        in_maps.append({"xT": xT, "wq": wq_c, "wk": wk_c, "wv": wv_c, "wp": wp_c})
    return in_maps


def _run(x, Wq, Wk, Wv, Wp, bp, b, t, d, cfg, trace=False):
    from concourse.bass_utils import run_bass_kernel_spmd
    nc = _get_nc(b, t, d, cfg)
    in_maps = _prepare_in_maps(x, Wq, Wk, Wv, Wp, b, t, d, H)
    res = run_bass_kernel_spmd(nc, in_maps, core_ids=list(range(NCORES)), trace=trace)
    acc = np.zeros((b * t, d), dtype=np.float64)
    for r in res.results:
        acc += r["out_p"].astype(np.float64)
    out = (acc + np.asarray(bp, dtype=np.float64)).astype(np.float32)
    return out.reshape(b, t, d), res


KERNEL_CFG = "f32r"


def kernel(x, Wq, Wk, Wv, Wp, bp):
    out, _ = _run(np.asarray(x), np.asarray(Wq), np.asarray(Wk), np.asarray(Wv),
                  np.asarray(Wp), np.asarray(bp), B, T, D, KERNEL_CFG, trace=False)
    return out
